# revision 5
# baseline (speedup 1.0000x reference)
"""MoE FFN with Sinkhorn (OT) routing — Trainium2 Bass kernel, 8 NeuronCores.

Strategy (expert-parallel):
  - Router (logits -> log-domain Sinkhorn -> top-2) runs on host in fp32
    numpy mirroring the reference ops; it is ~0.01% of the FLOPs.
  - Each of the 8 cores evaluates ONE expert's SwiGLU FFN over its assigned
    tokens (gather mode) or all tokens (dense mode), scales rows by the
    combine weight (column k of the transport plan for slot k), and the host
    scatter-adds the per-expert partials into the full output.
  - Device kernel per token-chunk of 512:
      phase A: g/u = x @ Wg^T, x @ Wu^T  (accumulate over d in PSUM),
               h = silu(g) * u  -> SBUF (f-major layout: 32 tiles (128f, 512tok))
      phase B: y = h^T @ Wd^T accumulated over all 32 f-tiles into 8 PSUM
               banks (4 tok-subtiles x 2 d-halves), evicted via ACT Copy with
               per-partition scale = combine weight, DMA'd out.
    Weights stream from HBM once per chunk in contiguous 0.5-1 MB blocks.
"""

import numpy as np

import concourse.bass as bass
import concourse.mybir as mybir
import concourse.tile as tile
from concourse.bass_utils import run_bass_kernel_spmd

# Problem constants (hardcoded per contract)
B, T, D, F, E = 2, 2048, 1024, 4096, 8
N = B * T                      # 4096 tokens
EPS = 0.05
N_ITERS = 20
TOP_K = 2

P = 128                        # partitions
NK = D // P                    # 8 d-tiles
NJ = F // P                    # 32 f-tiles
TOK_CHUNK = 512                # tokens per device chunk
N_CORES = 8

import os

GATHER = os.environ.get("MOE_GATHER", "1") == "1"
MM_DTYPE = {
    "f32": mybir.dt.float32,
    "f32r": mybir.dt.float32r,
}[os.environ.get("MOE_MM_DTYPE", "f32r")]

_f32 = np.float32


# ---------------------------------------------------------------- host router
def _logsumexp(a, axis):
    amax = np.max(a, axis=axis, keepdims=True)
    return np.log(np.sum(np.exp(a - amax), axis=axis, keepdims=True)) + amax


def _routing(xf, gate_W):
    """fp32 numpy mirror of the reference router. Returns (pi, top2)."""
    logits = xf @ gate_W.T                       # (N, E)
    la = (-logits) / _f32(EPS)
    for _ in range(N_ITERS):
        la = la - _logsumexp(la, axis=1)
        la = la - _logsumexp(la, axis=0)
    pi = np.exp(la)
    top2 = np.argsort(-pi, axis=1, kind="stable")[:, :TOP_K]
    return pi.astype(_f32), top2


# ---------------------------------------------------------------- device kernel
def _build_kernel(n_chunks: int):
    """One expert's SwiGLU over n_chunks*512 tokens. SPMD across 8 cores."""
    nc = bass.Bass(
        "TRN2", target_bir_lowering=False, debug=False, num_devices=N_CORES
    )
    f32 = mybir.dt.float32
    mmdt = MM_DTYPE            # matmul-operand tensors carry this dtype end-to-end
    n_tile = n_chunks * 4      # token tiles of 128

    xt_d = nc.declare_dram_parameter("xt", [n_chunks, P, NK, TOK_CHUNK], mmdt, isOutput=False)
    wgu_d = nc.declare_dram_parameter("wgu", [NJ, P, 2 * NK, P], mmdt, isOutput=False)
    wd_d = nc.declare_dram_parameter("wd", [NJ, P, D], mmdt, isOutput=False)
    wv_d = nc.declare_dram_parameter("wv", [P, n_tile], f32, isOutput=False)
    out_d = nc.declare_dram_parameter("out", [n_chunks * TOK_CHUNK, D], f32, isOutput=True)

    xt = xt_d.ap()
    wgu = wgu_d.ap()
    wd = wd_d.ap()
    wv = wv_d.ap()
    out = out_d.ap()

    with tile.TileContext(nc) as tc:
        with (
            tc.tile_pool(name="consts", bufs=1) as consts,
            tc.tile_pool(name="xpool", bufs=2) as xpool,
            tc.tile_pool(name="wpool", bufs=1) as wpool,
            tc.tile_pool(name="hpool", bufs=1) as hpool,
            tc.tile_pool(name="spool", bufs=2) as spool,
            tc.tile_pool(name="ypool", bufs=3) as ypool,
            tc.tile_pool(name="psum", bufs=8, space="PSUM") as psum,
        ):
            wv_sb = consts.tile([P, n_tile], f32)
            nc.sync.dma_start(out=wv_sb, in_=wv)

            for c in range(n_chunks):
                xt_sb = xpool.tile([P, NK, TOK_CHUNK], mmdt, tag="xt", name=f"xt{c}")
                nc.sync.dma_start(out=xt_sb, in_=xt[c])

                # ---- phase A: h = silu(x Wg^T) * (x Wu^T), f-major tiles
                h_tiles = []
                for j in range(NJ):
                    wgu_sb = wpool.tile(
                        [P, 2 * NK, P], mmdt, tag="wgu", bufs=3, name=f"wgu{c}_{j}"
                    )
                    nc.sync.dma_start(out=wgu_sb, in_=wgu[j])

                    pg = psum.tile([P, TOK_CHUNK], f32, tag="ps", name=f"pg{c}_{j}")
                    pu = psum.tile([P, TOK_CHUNK], f32, tag="ps", name=f"pu{c}_{j}")
                    for k in range(NK):
                        nc.tensor.matmul(
                            pg,
                            lhsT=wgu_sb[:, k, :],
                            rhs=xt_sb[:, k, :],
                            start=(k == 0),
                            stop=(k == NK - 1),
                        )
                    for k in range(NK):
                        nc.tensor.matmul(
                            pu,
                            lhsT=wgu_sb[:, NK + k, :],
                            rhs=xt_sb[:, k, :],
                            start=(k == 0),
                            stop=(k == NK - 1),
                        )
                    sil = spool.tile([P, TOK_CHUNK], f32, tag="sil", name=f"sil{c}_{j}")
                    nc.scalar.activation(
                        sil, pg, mybir.ActivationFunctionType.Silu
                    )
                    h = hpool.tile([P, TOK_CHUNK], mmdt, tag="h", bufs=NJ + 2, name=f"h{c}_{j}")
                    nc.vector.tensor_mul(h, sil, pu)
                    h_tiles.append(h)

                # ---- phase B: y[tok, d] = sum_f h[f, tok] * wd[f, d]
                py = [
                    psum.tile([P, TOK_CHUNK], f32, tag="ps", name=f"py{c}_{i}")
                    for i in range(8)
                ]
                for j in range(NJ):
                    wd_sb = wpool.tile([P, D], mmdt, tag="wd", bufs=4, name=f"wd{c}_{j}")
                    nc.sync.dma_start(out=wd_sb, in_=wd[j])
                    for s in range(4):
                        for dc in range(2):
                            nc.tensor.matmul(
                                py[s * 2 + dc],
                                lhsT=h_tiles[j][:, s * P : (s + 1) * P],
                                rhs=wd_sb[:, dc * 512 : (dc + 1) * 512],
                                start=(j == 0),
                                stop=(j == NJ - 1),
                            )
                for s in range(4):
                    tidx = c * 4 + s
                    ysb = ypool.tile([P, D], f32, tag="y", name=f"y{c}_{s}")
                    for dc in range(2):
                        nc.scalar.activation(
                            ysb[:, dc * 512 : (dc + 1) * 512],
                            py[s * 2 + dc],
                            mybir.ActivationFunctionType.Copy,
                            scale=wv_sb[:, tidx : tidx + 1],
                        )
                    nc.sync.dma_start(
                        out=out[tidx * P : (tidx + 1) * P, :], in_=ysb
                    )

    _split_multiwait_instructions(nc)
    return nc


def _split_multiwait_instructions(nc, max_waits: int = 1) -> int:
    """This walrus build rejects >2 sync waits per TPB_CTRL instruction (the
    TileContext tail Drain accumulates one wait per live semaphore). Move
    excess waits onto preceding single-wait EventSemaphore instructions on the
    same engine — same-engine program order preserves the semantics."""
    n_split = 0
    for f in nc.m.functions:
        for bb in f.blocks:
            new_insts = []
            for inst in bb.instructions:
                si = inst.sync_info
                if si is not None and si.on_wait and len(si.on_wait) > max_waits:
                    waits = list(si.on_wait)
                    extra, keep = waits[:-max_waits], waits[-max_waits:]
                    for i, w in enumerate(extra):
                        new_insts.append(
                            mybir.InstEventSemaphore(
                                name=f"{inst.name}-wsplit{i}",
                                opcode="EventSemaphore",
                                engine=inst.engine,
                                sync_info=mybir.SyncInfo(on_wait=[w], on_update=[]),
                            )
                        )
                        n_split += 1
                    inst.sync_info = mybir.SyncInfo(
                        on_wait=keep, on_update=list(si.on_update or [])
                    )
                new_insts.append(inst)
            bb.instructions[:] = new_insts
    return n_split


# ---------------------------------------------------------------- host prep
def _prep_core_inputs(xg, Wg, Wu, Wd, w_slot, n_chunks):
    """Pack one core's arrays into the DMA-friendly layouts the kernel expects."""
    C = n_chunks * TOK_CHUNK
    # xt[c, p, k, t] = xg[c*512 + t, k*128 + p]
    xt = np.ascontiguousarray(
        xg.reshape(n_chunks, TOK_CHUNK, NK, P).transpose(0, 3, 2, 1)
    )
    # wgu[j, p, kk, m]: kk<8 -> Wg[j*128+m, kk*128+p]; kk>=8 -> Wu[...]
    wg_r = Wg.reshape(NJ, P, NK, P).transpose(0, 3, 2, 1)   # [j, p, k, m]
    wu_r = Wu.reshape(NJ, P, NK, P).transpose(0, 3, 2, 1)
    wgu = np.ascontiguousarray(np.concatenate([wg_r, wu_r], axis=2))
    # wd[j, p, d] = Wd[d, j*128+p]
    wd = np.ascontiguousarray(Wd.transpose(1, 0).reshape(NJ, P, D))
    # wv[p, m] = w_slot[m*128 + p]
    wv = np.ascontiguousarray(w_slot.reshape(C // P, P).T)
    return {"xt": xt, "wgu": wgu, "wd": wd, "wv": wv}


_BUILT = {}


def _get_kernel(n_chunks):
    if n_chunks not in _BUILT:
        _BUILT[n_chunks] = _build_kernel(n_chunks)
    return _BUILT[n_chunks]


def kernel(x, gate_W, W_gate, W_up, W_down, _return_results=False, _run_kwargs=None):
    xf = np.ascontiguousarray(x.reshape(N, D)).astype(_f32, copy=False)
    gate_W = np.asarray(gate_W, dtype=_f32)
    pi, top2 = _routing(xf, gate_W)

    if GATHER:
        # token lists per expert with their combine weight (pi column k for slot k)
        tok_lists = [[] for _ in range(E)]
        wt_lists = [[] for _ in range(E)]
        for k in range(TOP_K):
            idx = top2[:, k]
            wk = pi[:, k]
            for e in range(E):
                sel = np.nonzero(idx == e)[0]
                tok_lists[e].append(sel)
                wt_lists[e].append(wk[sel])
        toks = [np.concatenate(t) for t in tok_lists]
        wts = [np.concatenate(w) for w in wt_lists]
        cap = max(len(t) for t in toks)
        n_chunks = max(1, -(-cap // TOK_CHUNK))
        C = n_chunks * TOK_CHUNK
        in_maps = []
        for e in range(E):
            xg = np.zeros((C, D), dtype=_f32)
            xg[: len(toks[e])] = xf[toks[e]]
            w_slot = np.zeros((C,), dtype=_f32)
            w_slot[: len(wts[e])] = wts[e]
            in_maps.append(
                _prep_core_inputs(
                    xg, W_gate[e], W_up[e], W_down[e], w_slot, n_chunks
                )
            )
    else:
        n_chunks = N // TOK_CHUNK
        C = N
        in_maps = []
        for e in range(E):
            w_slot = np.zeros((N,), dtype=_f32)
            for k in range(TOP_K):
                sel = top2[:, k] == e
                w_slot[sel] = pi[sel, k]
            in_maps.append(
                _prep_core_inputs(
                    xf, W_gate[e], W_up[e], W_down[e], w_slot, n_chunks
                )
            )

    nc = _get_kernel(n_chunks)
    res = run_bass_kernel_spmd(
        nc, in_maps, list(range(N_CORES)), **(_run_kwargs or {})
    )

    out_full = np.zeros((N, D), dtype=_f32)
    if GATHER:
        for e in range(E):
            ye = res.results[e]["out"]
            nt = len(toks[e])
            out_full[toks[e]] += ye[:nt]
    else:
        for e in range(E):
            out_full += res.results[e]["out"]

    out_full = out_full.reshape(B, T, D)
    if _return_results:
        return out_full, res
    return out_full


# revision 7
# speedup vs baseline: 1.0767x; 1.0767x over previous
"""MoE FFN with Sinkhorn (OT) routing — Trainium2 Bass kernel, 8 NeuronCores.

Strategy (expert-parallel):
  - Router (logits -> log-domain Sinkhorn -> top-2) runs on host in fp32
    numpy mirroring the reference ops; it is ~0.01% of the FLOPs.
  - Each of the 8 cores evaluates ONE expert's SwiGLU FFN over its assigned
    tokens (gather mode) or all tokens (dense mode), scales rows by the
    combine weight (column k of the transport plan for slot k), and the host
    scatter-adds the per-expert partials into the full output.
  - Device kernel per token-chunk of 512:
      phase A: g/u = x @ Wg^T, x @ Wu^T  (accumulate over d in PSUM),
               h = silu(g) * u  -> SBUF (f-major layout: 32 tiles (128f, 512tok))
      phase B: y = h^T @ Wd^T accumulated over all 32 f-tiles into 8 PSUM
               banks (4 tok-subtiles x 2 d-halves), evicted via ACT Copy with
               per-partition scale = combine weight, DMA'd out.
    Weights stream from HBM once per chunk in contiguous 0.5-1 MB blocks.
"""

import numpy as np

import concourse.bass as bass
import concourse.mybir as mybir
import concourse.tile as tile
from concourse.bass_utils import run_bass_kernel_spmd

# Problem constants (hardcoded per contract)
B, T, D, F, E = 2, 2048, 1024, 4096, 8
N = B * T                      # 4096 tokens
EPS = 0.05
N_ITERS = 20
TOP_K = 2

P = 128                        # partitions
NK = D // P                    # 8 d-tiles
NJ = F // P                    # 32 f-tiles
TOK_CHUNK = 512                # tokens per device chunk
N_CORES = 8

import os

GATHER = os.environ.get("MOE_GATHER", "1") == "1"
MM_DTYPE = {
    "f32": mybir.dt.float32,
    "f32r": mybir.dt.float32r,
}[os.environ.get("MOE_MM_DTYPE", "f32r")]

_f32 = np.float32


# ---------------------------------------------------------------- host router
def _logsumexp(a, axis):
    amax = np.max(a, axis=axis, keepdims=True)
    return np.log(np.sum(np.exp(a - amax), axis=axis, keepdims=True)) + amax


def _routing(xf, gate_W):
    """fp32 numpy mirror of the reference router. Returns (pi, top2)."""
    logits = xf @ gate_W.T                       # (N, E)
    la = (-logits) / _f32(EPS)
    for _ in range(N_ITERS):
        la = la - _logsumexp(la, axis=1)
        la = la - _logsumexp(la, axis=0)
    pi = np.exp(la)
    top2 = np.argsort(-pi, axis=1, kind="stable")[:, :TOP_K]
    return pi.astype(_f32), top2


# ---------------------------------------------------------------- device kernel
def _chunk_plan(cap: int) -> tuple:
    """Split `cap` token slots into chunks of <=512 (multiples of 128,
    min 256 so float32r matmuls stay at full rate)."""
    q, r = divmod(max(cap, 256), TOK_CHUNK)
    chunks = [TOK_CHUNK] * q
    if r:
        chunks.append(max(256, -(-r // P) * P))
    return tuple(chunks)


def _build_kernel(chunks: tuple):
    """One expert's SwiGLU over sum(chunks) tokens. SPMD across 8 cores."""
    nc = bass.Bass(
        "TRN2", target_bir_lowering=False, debug=False, num_devices=N_CORES
    )
    f32 = mybir.dt.float32
    mmdt = MM_DTYPE            # matmul-operand tensors carry this dtype end-to-end
    C = sum(chunks)
    n_tile = C // P            # token tiles of 128

    xt_d = nc.declare_dram_parameter("xt", [P, NK, C], mmdt, isOutput=False)
    wgu_d = nc.declare_dram_parameter("wgu", [NJ, P, 2 * NK, P], mmdt, isOutput=False)
    wd_d = nc.declare_dram_parameter("wd", [NJ, P, D], mmdt, isOutput=False)
    wv_d = nc.declare_dram_parameter("wv", [P, n_tile], f32, isOutput=False)
    out_d = nc.declare_dram_parameter("out", [C, D], f32, isOutput=True)

    xt = xt_d.ap()
    wgu = wgu_d.ap()
    wd = wd_d.ap()
    wv = wv_d.ap()
    out = out_d.ap()

    with tile.TileContext(nc) as tc:
        with (
            tc.tile_pool(name="consts", bufs=1) as consts,
            tc.tile_pool(name="xpool", bufs=2) as xpool,
            tc.tile_pool(name="wpool", bufs=1) as wpool,
            tc.tile_pool(name="hpool", bufs=1) as hpool,
            tc.tile_pool(name="spool", bufs=2) as spool,
            tc.tile_pool(name="ypool", bufs=3) as ypool,
            tc.tile_pool(name="psum", bufs=8, space="PSUM") as psum,
        ):
            wv_sb = consts.tile([P, n_tile], f32)
            nc.sync.dma_start(out=wv_sb, in_=wv)

            off = 0
            for c, cs in enumerate(chunks):
                n_sub = cs // P
                xt_sb = xpool.tile([P, NK, TOK_CHUNK], mmdt, tag="xt", name=f"xt{c}")
                nc.sync.dma_start(out=xt_sb[:, :, :cs], in_=xt[:, :, off : off + cs])

                # ---- phase A: h = silu(x Wg^T) * (x Wu^T), f-major tiles
                h_tiles = []
                for j in range(NJ):
                    wgu_sb = wpool.tile(
                        [P, 2 * NK, P], mmdt, tag="wgu", bufs=3, name=f"wgu{c}_{j}"
                    )
                    nc.sync.dma_start(out=wgu_sb, in_=wgu[j])

                    pg = psum.tile([P, TOK_CHUNK], f32, tag="ps", name=f"pg{c}_{j}")
                    pu = psum.tile([P, TOK_CHUNK], f32, tag="ps", name=f"pu{c}_{j}")
                    for k in range(NK):
                        nc.tensor.matmul(
                            pg[:, :cs],
                            lhsT=wgu_sb[:, k, :],
                            rhs=xt_sb[:, k, :cs],
                            start=(k == 0),
                            stop=(k == NK - 1),
                        )
                    for k in range(NK):
                        nc.tensor.matmul(
                            pu[:, :cs],
                            lhsT=wgu_sb[:, NK + k, :],
                            rhs=xt_sb[:, k, :cs],
                            start=(k == 0),
                            stop=(k == NK - 1),
                        )
                    sil = spool.tile([P, TOK_CHUNK], f32, tag="sil", name=f"sil{c}_{j}")
                    nc.scalar.activation(
                        sil[:, :cs], pg[:, :cs], mybir.ActivationFunctionType.Silu
                    )
                    h = hpool.tile([P, TOK_CHUNK], mmdt, tag="h", bufs=NJ + 2, name=f"h{c}_{j}")
                    nc.vector.tensor_mul(h[:, :cs], sil[:, :cs], pu[:, :cs])
                    h_tiles.append(h)

                # ---- phase B: y[tok, d] = sum_f h[f, tok] * wd[f, d]
                py = [
                    psum.tile([P, TOK_CHUNK], f32, tag="ps", name=f"py{c}_{i}")
                    for i in range(n_sub * 2)
                ]
                for j in range(NJ):
                    wd_sb = wpool.tile([P, D], mmdt, tag="wd", bufs=4, name=f"wd{c}_{j}")
                    nc.sync.dma_start(out=wd_sb, in_=wd[j])
                    for s in range(n_sub):
                        for dc in range(2):
                            nc.tensor.matmul(
                                py[s * 2 + dc][:, :512],
                                lhsT=h_tiles[j][:, s * P : (s + 1) * P],
                                rhs=wd_sb[:, dc * 512 : (dc + 1) * 512],
                                start=(j == 0),
                                stop=(j == NJ - 1),
                            )
                for s in range(n_sub):
                    tidx = off // P + s
                    ysb = ypool.tile([P, D], f32, tag="y", name=f"y{c}_{s}")
                    # split eviction across ACT and DVE to halve the latency
                    nc.scalar.activation(
                        ysb[:, 0:512],
                        py[s * 2][:, :512],
                        mybir.ActivationFunctionType.Copy,
                        scale=wv_sb[:, tidx : tidx + 1],
                    )
                    nc.vector.tensor_scalar_mul(
                        ysb[:, 512:1024],
                        py[s * 2 + 1][:, :512],
                        wv_sb[:, tidx : tidx + 1],
                    )
                    nc.sync.dma_start(
                        out=out[tidx * P : (tidx + 1) * P, :], in_=ysb
                    )
                off += cs

    _split_multiwait_instructions(nc)
    return nc


def _split_multiwait_instructions(nc, max_waits: int = 1) -> int:
    """This walrus build rejects >2 sync waits per TPB_CTRL instruction (the
    TileContext tail Drain accumulates one wait per live semaphore). Move
    excess waits onto preceding single-wait EventSemaphore instructions on the
    same engine — same-engine program order preserves the semantics."""
    n_split = 0
    for f in nc.m.functions:
        for bb in f.blocks:
            new_insts = []
            for inst in bb.instructions:
                si = inst.sync_info
                if si is not None and si.on_wait and len(si.on_wait) > max_waits:
                    waits = list(si.on_wait)
                    extra, keep = waits[:-max_waits], waits[-max_waits:]
                    for i, w in enumerate(extra):
                        new_insts.append(
                            mybir.InstEventSemaphore(
                                name=f"{inst.name}-wsplit{i}",
                                opcode="EventSemaphore",
                                engine=inst.engine,
                                sync_info=mybir.SyncInfo(on_wait=[w], on_update=[]),
                            )
                        )
                        n_split += 1
                    inst.sync_info = mybir.SyncInfo(
                        on_wait=keep, on_update=list(si.on_update or [])
                    )
                new_insts.append(inst)
            bb.instructions[:] = new_insts
    return n_split


# ---------------------------------------------------------------- host prep
def _prep_core_inputs(xg, Wg, Wu, Wd, w_slot):
    """Pack one core's arrays into the DMA-friendly layouts the kernel expects."""
    C = xg.shape[0]
    # xt[p, k, n] = xg[n, k*128 + p]
    xt = np.ascontiguousarray(xg.reshape(C, NK, P).transpose(2, 1, 0))
    # wgu[j, p, kk, m]: kk<8 -> Wg[j*128+m, kk*128+p]; kk>=8 -> Wu[...]
    wg_r = Wg.reshape(NJ, P, NK, P).transpose(0, 3, 2, 1)   # [j, p, k, m]
    wu_r = Wu.reshape(NJ, P, NK, P).transpose(0, 3, 2, 1)
    wgu = np.ascontiguousarray(np.concatenate([wg_r, wu_r], axis=2))
    # wd[j, p, d] = Wd[d, j*128+p]
    wd = np.ascontiguousarray(Wd.transpose(1, 0).reshape(NJ, P, D))
    # wv[p, m] = w_slot[m*128 + p]
    wv = np.ascontiguousarray(w_slot.reshape(C // P, P).T)
    return {"xt": xt, "wgu": wgu, "wd": wd, "wv": wv}


_BUILT = {}


def _get_kernel(chunks):
    if chunks not in _BUILT:
        _BUILT[chunks] = _build_kernel(chunks)
    return _BUILT[chunks]


def kernel(x, gate_W, W_gate, W_up, W_down, _return_results=False, _run_kwargs=None):
    xf = np.ascontiguousarray(x.reshape(N, D)).astype(_f32, copy=False)
    gate_W = np.asarray(gate_W, dtype=_f32)
    pi, top2 = _routing(xf, gate_W)

    if GATHER:
        # token lists per expert with their combine weight (pi column k for slot k)
        tok_lists = [[] for _ in range(E)]
        wt_lists = [[] for _ in range(E)]
        for k in range(TOP_K):
            idx = top2[:, k]
            wk = pi[:, k]
            for e in range(E):
                sel = np.nonzero(idx == e)[0]
                tok_lists[e].append(sel)
                wt_lists[e].append(wk[sel])
        toks = [np.concatenate(t) for t in tok_lists]
        wts = [np.concatenate(w) for w in wt_lists]
        cap = max(len(t) for t in toks)
        chunks = _chunk_plan(cap)
        C = sum(chunks)
        in_maps = []
        for e in range(E):
            xg = np.zeros((C, D), dtype=_f32)
            xg[: len(toks[e])] = xf[toks[e]]
            w_slot = np.zeros((C,), dtype=_f32)
            w_slot[: len(wts[e])] = wts[e]
            in_maps.append(
                _prep_core_inputs(xg, W_gate[e], W_up[e], W_down[e], w_slot)
            )
    else:
        chunks = _chunk_plan(N)
        C = N
        in_maps = []
        for e in range(E):
            w_slot = np.zeros((N,), dtype=_f32)
            for k in range(TOP_K):
                sel = top2[:, k] == e
                w_slot[sel] = pi[sel, k]
            in_maps.append(
                _prep_core_inputs(xf, W_gate[e], W_up[e], W_down[e], w_slot)
            )

    nc = _get_kernel(chunks)
    res = run_bass_kernel_spmd(
        nc, in_maps, list(range(N_CORES)), **(_run_kwargs or {})
    )

    out_full = np.zeros((N, D), dtype=_f32)
    if GATHER:
        for e in range(E):
            ye = res.results[e]["out"]
            nt = len(toks[e])
            out_full[toks[e]] += ye[:nt]
    else:
        for e in range(E):
            out_full += res.results[e]["out"]

    out_full = out_full.reshape(B, T, D)
    if _return_results:
        return out_full, res
    return out_full
